# revision 14
# baseline (speedup 1.0000x reference)
"""AmpNorm Trainium2 kernel: FFT-domain amplitude normalization (v4).

reference semantics:
    fft = fft2(x); amp = fftshift(|fft|); pha = angle(fft)
    amp_mean = mean(amp, axis=0)
    new_amp = (1-m)*running_amp + m*amp_mean     (EMA branch; init branch if sum==0)
    out = real(ifft2(ifftshift(new_amp) * exp(i*pha)))

Device formulation (per [512,512] image; shifts absorbed on host):
    Z = F @ X @ F computed for the k_c half-spectrum [0, 256] (x real =>
    Z Hermitian; the ratio is symmetrized host-side so the half determines
    the output exactly). P = Z/|Z| is stored IN SBUF (phase); out is rebuilt
    as ifft2 of numer * P where numer = ra_sym + mom * AllReduce(sum |Z|).

Stages (radix-2 split, every PE pass contracts <=256 rows):
  stage 1 (rows):   input butterflies precomputed ON HOST (xp tensor),
                    [B0|B0i]/[B1|B1i] merged-rhs matmuls.
  stage 2 (cols):   merged [Be_r|Be_i] (129 even k_c incl nyquist) and
                    [Bo_r|Bo_i] (128 odd k_c) matmuls.
  stage 3 (rows^-1): radix-2 DIT (Me/Mo on k_r parity chunks); E-part runs
                    early (gated on the even-k_r AllReduce half), O-part +
                    butterfly after the odd half.
  stage 4 (cols^-1): two-chunk contraction against Gw1/Gw2; nyquist rank-1
                    applied during PSUM evacuation via STT broadcast.

AllReduce is split per (channel, k_r-parity): 6 collectives of 131.5 KB,
so stage-3 E-work overlaps the odd-half wire time and the post-AR tail
shrinks. Bounce buffers are [128, 1028] partition-major (128 descriptors).
"""
import sys

sys.path.insert(0, "/opt/trn_rl_repo")

import numpy as np

N_CORES = 8
B, C, H, W = 32, 3, 512, 512
B_LOC = B // N_CORES          # 4 batches per core
N_IMG = B_LOC * C             # 12 images per core
NBLK = H // 128               # 4 partition blocks
KC = 257                      # half-spectrum cols: [evens 0..256 | odds]
NE = 129                      # even k_c count (incl nyquist)
NO = 128                      # odd k_c count
MOMENTUM = 0.1

# bf16 constants, packed column-wise into one [128, CST_COLS] tensor.
# name -> [rows, width]; order = DMA order (hot | cold).
CDEFS = {
    # hot -- stage1 merged rhs: CA = [B0r|B0i], CB = [B1r|B1i]
    "CA": [256, 512], "CB": [256, 512],
    # hot -- stage2 merged rhs (radix-2 DIF over n_c)
    "CE": [256, 2 * NE], "CEm": [256, 2 * NE],
    "CO": [256, 2 * NO], "COm": [256, 2 * NO],
    # cold -- pass-2 tables
    "MeRI": [256, 512], "MeIR": [256, 512],
    "MoRI": [256, 512], "MoIR": [256, 512],
    "Gw1": [256, W], "Gw2": [256, W],
    "GnR": [H, H], "GnI": [H, H],
    # (-1)^{n_c} broadcast row for the nyquist rank-1 evac fold
    "SGN": [128, W],
}


def _cst_layout():
    """name -> list of (col_offset, rows, width) per 128-row chunk."""
    chunks = {}
    off = 0
    for name, (r, wdt) in CDEFS.items():
        lst = []
        for p0 in range(0, r, 128):
            rows = min(128, r - p0)
            lst.append((off, rows, wdt))
            off += wdt
        chunks[name] = lst
    return chunks, off


_cached = {}


def _build():
    from concourse import bacc, tile, mybir

    f32 = mybir.dt.float32
    bf16 = mybir.dt.bfloat16
    Alu = mybir.AluOpType
    Act = mybir.ActivationFunctionType

    # Force every activation into the one table set covering
    # {copy, identity, square, ln, exp}: exactly one ACT table load.
    from concourse import hw_specs as _hw
    if not getattr(_hw, "_ampnorm_patched", False):
        _orig_get_tables = _hw.get_activation_tables

        def _patched(module_arch):
            tabs = _orig_get_tables(module_arch)
            keep = "natural_log_exp_and_others"
            covered = tabs[keep]
            return {
                name: (fns if name == keep else (fns - covered))
                for name, fns in tabs.items()
            }

        _hw.get_activation_tables = _patched
        _hw._ampnorm_patched = True
        import concourse.bacc as _bacc_mod
        _bacc_mod.get_activation_tables = _patched

    nc = bacc.Bacc("TRN2", target_bir_lowering=False, debug=False,
                   num_devices=N_CORES)

    xp_ext = nc.dram_tensor("xp", [B_LOC, C, 1024, 256], bf16,
                            kind="ExternalInput").ap()
    ra_ext = nc.dram_tensor("ra", [C, 128, NBLK * KC], bf16,
                            kind="ExternalInput").ap()
    mom_ext = nc.dram_tensor("mom", [128, 1], f32, kind="ExternalInput").ap()
    cchunks, CST_COLS = _cst_layout()
    cst_ext = nc.dram_tensor("CST", [128, CST_COLS], bf16,
                             kind="ExternalInput").ap()
    out_ext = nc.dram_tensor("out", [B_LOC, C, H, W], f32, kind="ExternalOutput").ap()

    h1 = cchunks["CE"][0][0]            # end of CB
    h2 = cchunks["MeRI"][0][0]          # end of COm (hot/cold split)

    q_slices = [(0, 128), (NE, KC)]     # stage3/4 k_c blocks

    with tile.TileContext(nc) as tc:
        with (
            tc.tile_pool(name="const", bufs=1) as constp,
            tc.tile_pool(name="zpool", bufs=1) as zp,
            tc.tile_pool(name="stage", bufs=1) as stgp,
            tc.tile_pool(name="stage2", bufs=2) as stg2p,
            tc.tile_pool(name="work", bufs=2) as workp,
            tc.tile_pool(name="work1", bufs=1) as w1p,
            tc.tile_pool(name="eripool", bufs=5) as erip,
            tc.tile_pool(name="psA", bufs=2, space="PSUM") as psA,
            tc.tile_pool(name="psB", bufs=2, space="PSUM") as psB,
            tc.tile_pool(name="dram", bufs=1, space="DRAM") as dramp,
        ):
            # ---- constants: ONE hot DMA (128 big descriptors), cold on
            # the gpsimd queue; mom/ra after hot on sync ----
            cbig = constp.tile([128, CST_COLS], bf16, name="cbig")
            nc.sync.dma_start(cbig[:, 0:h1], cst_ext[:, 0:h1])
            nc.sync.dma_start(cbig[:, h1:h2], cst_ext[:, h1:h2])
            nc.gpsimd.dma_start(cbig[:, h2:], cst_ext[:, h2:])
            mom_t = constp.tile([128, 1], f32, name="mom_t")
            nc.sync.dma_start(mom_t[:], mom_ext[:, :])
            eps_t = constp.tile([128, 1], f32, name="eps_t")
            nc.gpsimd.memset(eps_t[:], 1e-30)
            rat = {}
            cst = {
                name: [cbig[0:rows, o:o + wdt] for (o, rows, wdt) in lst]
                for name, lst in cchunks.items()
            }

            # ---- dummy warmup AllReduce: absorbs the CC barrier and the
            # first-collective warmup penalty before the real ARs ----
            wu_in = dramp.tile([128, 64], bf16, name="wu_in")
            wu_out = dramp.tile([128, 64], bf16, name="wu_out",
                                addr_space="Shared")
            wu_s = constp.tile([128, 64], bf16, name="wu_s")
            nc.gpsimd.memset(wu_s[:], 0.0)
            with tc.high_priority():
                nc.gpsimd.dma_start(wu_in[:], wu_s[:])
                nc.gpsimd.collective_compute(
                    "AllReduce", Alu.add,
                    replica_groups=[list(range(N_CORES))],
                    ins=[wu_in.opt()], outs=[wu_out.opt()])

            # ---- collective bounces, partition-major [128, 2*KC] bf16 ----
            ar_in = {}
            ar_out = {}
            for c in range(C):
                for par in range(2):
                    ar_in[(c, par)] = dramp.tile(
                        [128, 2 * KC], bf16, name=f"ar_in_{c}_{par}")
                    ar_out[(c, par)] = dramp.tile(
                        [128, 2 * KC], bf16, name=f"ar_out_{c}_{par}",
                        addr_space="Shared")

            def blocked(ap):  # [m*128+p, j] dram view -> [p, m, j]
                return ap.rearrange("(m p) j -> p m j", p=128)

            Zt = {}         # (c, b) -> SBUF phase tile [128, 2, 4, KC]
            amp_t = {}      # accumulator chain for current channel
            tny = {}        # c -> [128, 4, 4] f32 nyquist vectors

            # ===== PASS 1 (per channel): forward + amp accumulation =====
            def emit_p1(c):
                for b in range(B_LOC):
                    xq = nc.scalar if (c == 0 and b < 3) else nc.sync
                    # host-precomputed radix-2 butterflies: rows (key,ch,p)
                    xp = workp.tile([128, 8, 256], bf16, name="xp", tag="xp")
                    xq.dma_start(xp[:], xp_ext[b, c].rearrange(
                        "(k p) j -> p k j", p=128))
                    # stage 1: 4 merged psum groups [128, 2(mp), 512]
                    ups = {}
                    for gi, (key, kidx, cn) in enumerate((
                        ("pE", 0, "CA"), ("pO", 1, "CB"),
                        ("mE", 2, "CA"), ("mO", 3, "CB"),
                    )):
                        pool = psA if gi % 2 == 0 else psB
                        ps = pool.tile([128, 2, 512], f32, name=f"ps1{key}",
                                       tag="pA" if gi % 2 == 0 else "pB")
                        for mp in range(2):
                            mps = slice(mp * 128, (mp + 1) * 128)
                            for ch in range(2):
                                nc.tensor.matmul(ps[:, mp, :],
                                                 xp[:, kidx * 2 + ch, mps],
                                                 cst[cn][ch][:],
                                                 start=(ch == 0), stop=(ch == 1))
                        u = stg2p.tile([128, 2, 512], bf16, name=f"u{key}",
                                       tag=f"u{key}")
                        if gi % 2 == 0:
                            nc.vector.tensor_copy(u[:], ps[:])
                        else:
                            nc.scalar.copy(u[:], ps[:])
                        ups[key] = u
                    # stage 2: per mh, merged [128, 2(dm), 512] psums
                    z = zp.tile([128, 2, NBLK, KC], bf16, name=f"z{c}{b}",
                                tag=f"z{c}{b}")
                    for mh in range(2):
                        sfx = "E" if mh == 0 else "O"
                        pze = psA.tile([128, 2, 512], f32, name="pze", tag="pA")
                        pzo = psB.tile([128, 2, 512], f32, name="pzo", tag="pB")
                        for dm in range(2):
                            mr = slice(dm * 128, dm * 128 + 128)
                            mi = slice(256 + dm * 128, 256 + dm * 128 + 128)
                            for mp in range(2):
                                nc.tensor.matmul(pze[:, dm, 0:2 * NE],
                                                 ups["p" + sfx][:, mp, mr],
                                                 cst["CE"][mp][:],
                                                 start=(mp == 0), stop=False)
                            for mp in range(2):
                                nc.tensor.matmul(pze[:, dm, 0:2 * NE],
                                                 ups["p" + sfx][:, mp, mi],
                                                 cst["CEm"][mp][:],
                                                 start=False, stop=(mp == 1))
                            for mp in range(2):
                                nc.tensor.matmul(pzo[:, dm, 0:2 * NO],
                                                 ups["m" + sfx][:, mp, mr],
                                                 cst["CO"][mp][:],
                                                 start=(mp == 0), stop=False)
                            for mp in range(2):
                                nc.tensor.matmul(pzo[:, dm, 0:2 * NO],
                                                 ups["m" + sfx][:, mp, mi],
                                                 cst["COm"][mp][:],
                                                 start=False, stop=(mp == 1))
                        ms2 = slice(2 * mh, 2 * mh + 2)
                        nc.vector.tensor_copy(z[:, 0, ms2, 0:NE], pze[:, :, 0:NE])
                        nc.scalar.copy(z[:, 1, ms2, 0:NE], pze[:, :, NE:2 * NE])
                        nc.vector.tensor_copy(z[:, 0, ms2, NE:KC], pzo[:, :, 0:NO])
                        nc.scalar.copy(z[:, 1, ms2, NE:KC], pzo[:, :, NO:2 * NO])
                    # amp chain, image-wide [128, 4, KC]
                    sq1 = w1p.tile([128, NBLK, KC], bf16, name="sq1", tag="sq1")
                    nc.gpsimd.tensor_tensor(sq1[:], z[:, 0], z[:, 0], Alu.mult)
                    sq2 = w1p.tile([128, NBLK, KC], bf16, name="sq2", tag="sq2")
                    nc.vector.tensor_tensor(sq2[:], z[:, 1], z[:, 1], Alu.mult)
                    a2 = workp.tile([128, NBLK, KC], bf16, name="a2", tag="a2")
                    nc.vector.tensor_tensor(a2[:], sq1[:], sq2[:], Alu.add)
                    lna = w1p.tile([128, NBLK, KC], f32, name="lna", tag="lna")
                    nc.scalar.activation(lna[:], a2[:], Act.Ln, bias=eps_t[:, 0:1])
                    ampb = workp.tile([128, NBLK, KC], bf16, name="amp",
                                      tag="amp")
                    nc.scalar.activation(ampb[:], lna[:], Act.Exp, scale=0.5)
                    inv = workp.tile([128, NBLK, KC], bf16, name="inv", tag="inv")
                    nc.scalar.activation(inv[:], lna[:], Act.Exp, scale=-0.5)
                    if b == 0:
                        amp_t["acc"] = ampb
                    elif b < B_LOC - 1:
                        acc = workp.tile([128, NBLK, KC], bf16, name="acc",
                                         tag="acc")
                        nc.gpsimd.tensor_tensor(acc[:], amp_t["acc"][:], ampb[:],
                                                Alu.add)
                        amp_t["acc"] = acc
                    else:
                        acast = w1p.tile([128, NBLK, KC], bf16, name=f"ac{c}",
                                         tag=f"ac{c}")
                        nc.vector.tensor_tensor(acast[:], amp_t["acc"][:],
                                                ampb[:], Alu.add)
                    # phase in place: Z *= 1/|Z|
                    nc.vector.tensor_tensor(z[:, 0], z[:, 0], inv[:], Alu.mult)
                    nc.vector.tensor_tensor(z[:, 1], z[:, 1], inv[:], Alu.mult)
                    Zt[(c, b)] = z
                with tc.high_priority():
                    for par in range(2):
                        nc.scalar.dma_start(
                            ar_in[(c, par)][:].rearrange("p (m j) -> p m j", m=2),
                            acast[:, 2 * par:2 * par + 2, :])
                        nc.gpsimd.collective_compute(
                            "AllReduce",
                            Alu.add,
                            replica_groups=[list(range(N_CORES))],
                            ins=[ar_in[(c, par)].opt()],
                            outs=[ar_out[(c, par)].opt()],
                        )

            # ===== PASS 2 (per channel): ratio + inverse =====
            def emit_p2(c):
                fe, fo = p2_floors[c]
                stk = tc.tile_wait_until(fe)
                stk.__enter__()
                rat[c] = constp.tile([128, NBLK, KC], bf16, name=f"rat{c}")
                nc.sync.dma_start(
                    rat[c][:].rearrange("p m j -> p (m j)"), ra_ext[c])
                # --- even-k_r phase: gated on AR (c, 0) only ---
                red_e = w1p.tile([128, 2, KC], bf16, name="red_e", tag="red_e")
                nc.sync.dma_start(
                    red_e[:], ar_out[(c, 0)][:].rearrange("p (m j) -> p m j", m=2))
                num_e = stgp.tile([128, 2, KC], bf16, name=f"nume_{c}",
                                  tag=f"nume_{c}")
                nc.vector.scalar_tensor_tensor(
                    num_e[:], red_e[:], mom_t[:, 0:1], rat[c][:, 0:2, :],
                    op0=Alu.mult, op1=Alu.add)
                twr_e = {}
                twi_e = {}
                eris = {}
                for b in range(B_LOC):
                    z = Zt[(c, b)]
                    tre = workp.tile([128, 2, KC], bf16, name="twre", tag="twre")
                    tie = workp.tile([128, 2, KC], bf16, name="twie", tag="twie")
                    nc.vector.tensor_tensor(tre[:], z[:, 0, 0:2, :], num_e[:],
                                            Alu.mult)
                    nc.vector.tensor_tensor(tie[:], z[:, 1, 0:2, :], num_e[:],
                                            Alu.mult)
                    twr_e[b], twi_e[b] = tre, tie
                    # stage-3 E chains for both q-slices, evac to SBUF
                    eb = []
                    for qi, (q0, q1) in enumerate(q_slices):
                        qs = slice(q0, q1)
                        pool = psA if qi == 0 else psB
                        pse = pool.tile([128, 512], f32, name="pse",
                                        tag="pA" if qi == 0 else "pB")
                        for k in range(2):
                            nc.tensor.matmul(pse[:], tre[:, k, qs],
                                             cst["MeRI"][k][:],
                                             start=(k == 0), stop=False)
                        for k in range(2):
                            nc.tensor.matmul(pse[:], tie[:, k, qs],
                                             cst["MeIR"][k][:],
                                             start=False, stop=(k == 1))
                        eri = erip.tile([128, 512], bf16, name="eri",
                                        tag=f"eri{qi}")
                        nc.scalar.copy(eri[:], pse[:])
                        eb.append(eri)
                    eris[b] = eb
                stk.__exit__(None, None, None)
                stk = tc.tile_wait_until(fo)
                stk.__enter__()
                # --- odd-k_r phase: gated on AR (c, 1) ---
                red_o = w1p.tile([128, 2, KC], bf16, name="red_o", tag="red_o")
                nc.sync.dma_start(
                    red_o[:], ar_out[(c, 1)][:].rearrange("p (m j) -> p m j", m=2))
                num_o = stgp.tile([128, 2, KC], bf16, name=f"numo_{c}",
                                  tag=f"numo_{c}")
                nc.vector.scalar_tensor_tensor(
                    num_o[:], red_o[:], mom_t[:, 0:1], rat[c][:, 2:4, :],
                    op0=Alu.mult, op1=Alu.add)
                # nyquist side: wn = P[:, :, :, nyq] * numer_nyq
                nq2 = w1p.tile([128, 2, 2, NBLK // 2, 1], bf16, name="nq2",
                               tag="nq2")
                for t in range(2):
                    nc.scalar.copy(nq2[:, 0, t], num_e[:, :, 128:129])
                    nc.scalar.copy(nq2[:, 1, t], num_o[:, :, 128:129])
                wn = w1p.tile([128, B_LOC, 2, NBLK, 1], bf16, name="wn", tag="wn")
                for b in range(B_LOC):
                    zs = Zt[(c, b)][:, :, :, 128:129]   # [128, 2(t), 4(m), 1]
                    nc.vector.tensor_tensor(wn[:, b, :, 0:2], zs[:, :, 0:2],
                                            nq2[:, 0], Alu.mult)
                    nc.vector.tensor_tensor(wn[:, b, :, 2:4], zs[:, :, 2:4],
                                            nq2[:, 1], Alu.mult)
                psn = psA.tile([128, NBLK, B_LOC], f32, name="psn", tag="pA")
                for m in range(NBLK):
                    ms = slice(m * 128, (m + 1) * 128)
                    for ti, cn in enumerate(("GnR", "GnI")):
                        for k in range(NBLK):
                            nc.tensor.matmul(
                                psn[:, m, :], cst[cn][k][:, ms],
                                wn[:, :, ti, k, 0],
                                start=(ti == 0 and k == 0),
                                stop=(ti == 1 and k == NBLK - 1))
                tnyT = stgp.tile([128, NBLK, B_LOC], f32, name=f"tny{c}",
                                 tag=f"tny{c}")
                nc.scalar.copy(tnyT[:], psn[:])
                tny[c] = tnyT
                for b in range(B_LOC):
                    z = Zt[(c, b)]
                    two = workp.tile([128, 2, KC], bf16, name="twro", tag="twro")
                    tio = workp.tile([128, 2, KC], bf16, name="twio", tag="twio")
                    nc.vector.tensor_tensor(two[:], z[:, 0, 2:4, :], num_o[:],
                                            Alu.mult)
                    nc.vector.tensor_tensor(tio[:], z[:, 1, 2:4, :], num_o[:],
                                            Alu.mult)
                    # stage-3 O chains + DIT butterfly vs the stored E part
                    t2 = []
                    for qi, (q0, q1) in enumerate(q_slices):
                        qs = slice(q0, q1)
                        pool = psA if qi == 0 else psB
                        pso3 = pool.tile([128, 512], f32, name="pso3",
                                         tag="pA" if qi == 0 else "pB")
                        for k in range(2):
                            nc.tensor.matmul(pso3[:], two[:, k, qs],
                                             cst["MoRI"][k][:],
                                             start=(k == 0), stop=False)
                        for k in range(2):
                            nc.tensor.matmul(pso3[:], tio[:, k, qs],
                                             cst["MoIR"][k][:],
                                             start=False, stop=(k == 1))
                        t2q = stg2p.tile([128, 2, 512], bf16, name=f"t2_{qi}",
                                         tag=f"t2_{qi}")
                        nc.vector.tensor_tensor(t2q[:, 0, :], eris[b][qi][:],
                                                pso3[:], Alu.add)
                        nc.vector.tensor_tensor(t2q[:, 1, :], eris[b][qi][:],
                                                pso3[:], Alu.subtract)
                        t2.append(t2q)
                    # stage 4 + nyquist rank-1 evac fold
                    ostg = workp.tile([128, NBLK, W], f32, name="ostg", tag="ostg")
                    for half in range(2):
                        pool = psA if half == 0 else psB
                        pso = pool.tile([128, 2, 512], f32, name="pso",
                                        tag="pA" if half == 0 else "pB")
                        for j in range(2):
                            m = 2 * half + j
                            msh = slice((m % 2) * 128, (m % 2) * 128 + 128)
                            msi = slice(256 + (m % 2) * 128,
                                        256 + (m % 2) * 128 + 128)
                            nc.tensor.matmul(pso[:, j, :], t2[0][:, half, msh],
                                             cst["Gw1"][0][:],
                                             start=True, stop=False)
                            nc.tensor.matmul(pso[:, j, :], t2[1][:, half, msh],
                                             cst["Gw1"][1][:],
                                             start=False, stop=False)
                            nc.tensor.matmul(pso[:, j, :], t2[0][:, half, msi],
                                             cst["Gw2"][0][:],
                                             start=False, stop=False)
                            nc.tensor.matmul(pso[:, j, :], t2[1][:, half, msi],
                                             cst["Gw2"][1][:],
                                             start=False, stop=True)
                        for j in range(2):
                            m = 2 * half + j
                            nc.vector.scalar_tensor_tensor(
                                ostg[:, m, :], cst["SGN"][0][:],
                                tny[c][:, m, b:b + 1], pso[:, j, :],
                                op0=Alu.mult, op1=Alu.add)
                    nc.gpsimd.dma_start(blocked(out_ext[b, c]), ostg[:])
                stk.__exit__(None, None, None)

            emit_p1(0)
            emit_p1(1)
            emit_p1(2)
            # virtual-time floors: park AR-gated pass-2 work after pass-1 in
            # every engine queue (the scheduler otherwise interleaves it early
            # and the real AllReduce latency head-of-line-blocks pass-1)
            p2_floors = {0: (0.135, 0.150), 1: (0.165, 0.180), 2: (0.210, 0.225)}
            emit_p2(0)
            emit_p2(1)
            emit_p2(2)
    nc.compile()
    return nc


def _host_inputs(x, running_amp):
    import ml_dtypes
    bf16 = ml_dtypes.bfloat16

    j = np.arange(H)
    theta = -2.0 * np.pi * np.outer(j, j) / H
    Fc = np.exp(1j * theta)           # F[n,k] = w^{nk}
    Gc = np.conj(Fc)                  # G[n,k] = w^{-nk}

    # stage1: B0/B1 = even/odd columns of F, top 256 rows; merged [r|i]
    B0 = Fc[0:256, 0::2]
    B1 = Fc[0:256, 1::2]
    CA = np.concatenate([B0.real, B0.imag], axis=1).astype(bf16)
    CB = np.concatenate([B1.real, B1.imag], axis=1).astype(bf16)

    # stage2: radix-2 DIF over n_c
    n = np.arange(256)[:, None]
    ke = np.arange(NE)[None, :]
    ko = np.arange(NO)[None, :]
    Be = np.exp(-2j * np.pi * n * (2 * ke) / H)
    Bo = np.exp(-2j * np.pi * n * (2 * ko + 1) / H)
    CE = np.concatenate([Be.real, Be.imag], axis=1).astype(bf16)
    CEm = np.concatenate([-Be.imag, Be.real], axis=1).astype(bf16)
    CO = np.concatenate([Bo.real, Bo.imag], axis=1).astype(bf16)
    COm = np.concatenate([-Bo.imag, Bo.real], axis=1).astype(bf16)

    # stage3 radix-2 DIT: even/odd columns of G restricted per derivation
    n256 = np.arange(256)
    Me = np.exp(2j * np.pi * np.outer(n256, n256) / 256.0)
    Mo = np.exp(2j * np.pi * np.outer(2 * n256 + 1, n256) / 512.0)
    MeRI = np.concatenate([Me.real, Me.imag], axis=1).astype(bf16)
    MeIR = np.concatenate([-Me.imag, Me.real], axis=1).astype(bf16)
    MoRI = np.concatenate([Mo.real, Mo.imag], axis=1).astype(bf16)
    MoIR = np.concatenate([-Mo.imag, Mo.real], axis=1).astype(bf16)

    # k_c column order: [evens incl nyq | odds]
    cols = np.concatenate([np.arange(0, 257, 2), np.arange(1, 256, 2)])
    cols_noq = np.concatenate([np.arange(0, 256, 2), np.arange(1, 256, 2)])

    # stage-4 weights: w=1 for k_c=0, else 2 (nyquist handled separately)
    wgt = np.where(cols_noq == 0, 1.0, 2.0)
    Gsel = Gc[cols_noq, :]
    Gw1 = (wgt[:, None] * Gsel.real / (H * W)).astype(bf16)
    Gw2 = (-wgt[:, None] * Gsel.imag / (H * W)).astype(bf16)

    perm_kr = np.concatenate([np.arange(0, H, 2), np.arange(1, H, 2)])
    # nyquist path: T2nyq = sum_kr G[n_r,k_r] W[k_r]; G symmetric; rows in
    # k_r-perm order; 1/N^2 and w=1 folded in. Re only: Gr*Wr - Gi*Wi.
    Gn = Gc[perm_kr, :]
    GnR = (Gn.real / (H * W)).astype(bf16)
    GnI = (-Gn.imag / (H * W)).astype(bf16)
    sgnrow = np.where(np.arange(W) % 2 == 0, 1.0, -1.0)
    SGN = np.broadcast_to(sgnrow, (128, W)).astype(bf16)

    cvals = {
        "CA": CA, "CB": CB, "CE": CE, "CEm": CEm, "CO": CO, "COm": COm,
        "MeRI": MeRI, "MeIR": MeIR, "MoRI": MoRI, "MoIR": MoIR,
        "Gw1": Gw1, "Gw2": Gw2, "GnR": GnR, "GnI": GnI, "SGN": SGN,
    }

    cchunks, cst_cols = _cst_layout()
    CST = np.zeros((128, cst_cols), bf16)
    for name, lst in cchunks.items():
        arr = cvals[name]
        p0 = 0
        for (o, rows, wdt) in lst:
            CST[0:rows, o:o + wdt] = arr[p0:p0 + rows, :]
            p0 += rows
    consts = {"CST": CST}

    if abs(float(running_amp.sum())) == 0.0:
        ra_half = np.zeros((C, H, KC), np.float32)
        mom_eff = 1.0 / B
    else:
        ra_s = np.fft.ifftshift(running_amp, axes=(-2, -1)).astype(np.float64)
        ra_rev = ra_s[:, (-np.arange(H)) % H][:, :, (-np.arange(W)) % W]
        ra_sym = (1.0 - MOMENTUM) * 0.5 * (ra_s + ra_rev)
        ra_half = ra_sym[:, perm_kr][:, :, cols].astype(np.float32)
        mom_eff = MOMENTUM / B
    mom = np.full((128, 1), mom_eff, np.float32)

    # host-side radix-2 DIF butterflies over both axes (device stage-1 lhsT):
    # rows (key, chunk, p) with key order [p0, p1, m0, m1]
    y0 = x[:, :, 0:256, :] + x[:, :, 256:512, :]
    y1 = x[:, :, 0:256, :] - x[:, :, 256:512, :]
    p0 = y0[..., 0:256] + y0[..., 256:512]
    m0 = y0[..., 0:256] - y0[..., 256:512]
    p1 = y1[..., 0:256] + y1[..., 256:512]
    m1 = y1[..., 0:256] - y1[..., 256:512]
    XP = np.stack([p0, p1, m0, m1], axis=2).reshape(B, C, 1024, 256).astype(bf16)

    # ra in partition-major layout [C, 128, NBLK*KC] (fewer DMA descriptors)
    ra_pm = ra_half.reshape(C, NBLK, 128, KC).transpose(0, 2, 1, 3).reshape(
        C, 128, NBLK * KC)

    in_maps = []
    for i in range(N_CORES):
        m = {"xp": np.ascontiguousarray(XP[i * B_LOC:(i + 1) * B_LOC]),
             "ra": ra_pm.astype(bf16), "mom": mom}
        m.update(consts)
        in_maps.append(m)
    return in_maps


def kernel(x: np.ndarray, running_amp: np.ndarray) -> np.ndarray:
    from concourse.bass_utils import run_bass_kernel_spmd

    if "nc" not in _cached:
        _cached["nc"] = _build()
    nc = _cached["nc"]
    in_maps = _host_inputs(np.asarray(x, np.float32),
                           np.asarray(running_amp, np.float32))
    res = run_bass_kernel_spmd(nc, in_maps, list(range(N_CORES)))
    out = np.concatenate([res.results[i]["out"] for i in range(N_CORES)], axis=0)
    return out.astype(np.float32)


# revision 15
# speedup vs baseline: 1.1142x; 1.1142x over previous
"""AmpNorm Trainium2 kernel: FFT-domain amplitude normalization (v4).

reference semantics:
    fft = fft2(x); amp = fftshift(|fft|); pha = angle(fft)
    amp_mean = mean(amp, axis=0)
    new_amp = (1-m)*running_amp + m*amp_mean     (EMA branch; init branch if sum==0)
    out = real(ifft2(ifftshift(new_amp) * exp(i*pha)))

Device formulation (per [512,512] image; shifts absorbed on host):
    Z = F @ X @ F computed for the k_c half-spectrum [0, 256] (x real =>
    Z Hermitian; the ratio is symmetrized host-side so the half determines
    the output exactly). P = Z/|Z| is stored IN SBUF (phase); out is rebuilt
    as ifft2 of numer * P where numer = ra_sym + mom * AllReduce(sum |Z|).

Stages (radix-2 split, every PE pass contracts <=256 rows):
  stage 1 (rows):   input butterflies precomputed ON HOST (xp tensor),
                    [B0|B0i]/[B1|B1i] merged-rhs matmuls.
  stage 2 (cols):   merged [Be_r|Be_i] (129 even k_c incl nyquist) and
                    [Bo_r|Bo_i] (128 odd k_c) matmuls.
  stage 3 (rows^-1): radix-2 DIT (Me/Mo on k_r parity chunks); E-part runs
                    early (gated on the even-k_r AllReduce half), O-part +
                    butterfly after the odd half.
  stage 4 (cols^-1): two-chunk contraction against Gw1/Gw2; nyquist rank-1
                    applied during PSUM evacuation via STT broadcast.

AllReduce is split per (channel, k_r-parity): 6 collectives of 131.5 KB,
so stage-3 E-work overlaps the odd-half wire time and the post-AR tail
shrinks. Bounce buffers are [128, 1028] partition-major (128 descriptors).
"""
import sys

sys.path.insert(0, "/opt/trn_rl_repo")

import numpy as np

N_CORES = 8
B, C, H, W = 32, 3, 512, 512
B_LOC = B // N_CORES          # 4 batches per core
N_IMG = B_LOC * C             # 12 images per core
NBLK = H // 128               # 4 partition blocks
KC = 257                      # half-spectrum cols: [evens 0..256 | odds]
NE = 129                      # even k_c count (incl nyquist)
NO = 128                      # odd k_c count
MOMENTUM = 0.1

# bf16 constants, packed column-wise into one [128, CST_COLS] tensor.
# name -> [rows, width]; order = DMA order (hot | cold).
CDEFS = {
    # hot -- stage1 merged rhs: CA = [B0r|B0i], CB = [B1r|B1i]
    "CA": [256, 512], "CB": [256, 512],
    # hot -- stage2 merged rhs (radix-2 DIF over n_c)
    "CE": [256, 2 * NE], "CEm": [256, 2 * NE],
    "CO": [256, 2 * NO], "COm": [256, 2 * NO],
    # cold -- pass-2 tables
    "MeRI": [256, 512], "MeIR": [256, 512],
    "MoRI": [256, 512], "MoIR": [256, 512],
    "Gw1": [256, W], "Gw2": [256, W],
    "GnR": [H, H], "GnI": [H, H],
    # (-1)^{n_c} broadcast row for the nyquist rank-1 evac fold
    "SGN": [128, W],
}


def _cst_layout():
    """name -> list of (col_offset, rows, width) per 128-row chunk."""
    chunks = {}
    off = 0
    for name, (r, wdt) in CDEFS.items():
        lst = []
        for p0 in range(0, r, 128):
            rows = min(128, r - p0)
            lst.append((off, rows, wdt))
            off += wdt
        chunks[name] = lst
    return chunks, off


_cached = {}


def _build():
    from concourse import bacc, tile, mybir

    f32 = mybir.dt.float32
    bf16 = mybir.dt.bfloat16
    Alu = mybir.AluOpType
    Act = mybir.ActivationFunctionType

    # Force every activation into the one table set covering
    # {copy, identity, square, ln, exp}: exactly one ACT table load.
    from concourse import hw_specs as _hw
    if not getattr(_hw, "_ampnorm_patched", False):
        _orig_get_tables = _hw.get_activation_tables

        def _patched(module_arch):
            tabs = _orig_get_tables(module_arch)
            keep = "natural_log_exp_and_others"
            covered = tabs[keep]
            return {
                name: (fns if name == keep else (fns - covered))
                for name, fns in tabs.items()
            }

        _hw.get_activation_tables = _patched
        _hw._ampnorm_patched = True
        import concourse.bacc as _bacc_mod
        _bacc_mod.get_activation_tables = _patched

    nc = bacc.Bacc("TRN2", target_bir_lowering=False, debug=False,
                   num_devices=N_CORES)

    xp_ext = nc.dram_tensor("xp", [B_LOC, C, 1024, 256], bf16,
                            kind="ExternalInput").ap()
    ra_ext = nc.dram_tensor("ra", [C, 128, NBLK * KC], bf16,
                            kind="ExternalInput").ap()
    mom_ext = nc.dram_tensor("mom", [128, 1], f32, kind="ExternalInput").ap()
    cchunks, CST_COLS = _cst_layout()
    cst_ext = nc.dram_tensor("CST", [128, CST_COLS], bf16,
                             kind="ExternalInput").ap()
    out_ext = nc.dram_tensor("out", [B_LOC, C, H, W], f32, kind="ExternalOutput").ap()

    h1 = cchunks["CE"][0][0]            # end of CB
    h2 = cchunks["MeRI"][0][0]          # end of COm (hot/cold split)

    q_slices = [(0, 128), (NE, KC)]     # stage3/4 k_c blocks

    with tile.TileContext(nc) as tc:
        with (
            tc.tile_pool(name="const", bufs=1) as constp,
            tc.tile_pool(name="zpool", bufs=1) as zp,
            tc.tile_pool(name="stage", bufs=1) as stgp,
            tc.tile_pool(name="stage2", bufs=2) as stg2p,
            tc.tile_pool(name="work", bufs=2) as workp,
            tc.tile_pool(name="work1", bufs=1) as w1p,
            tc.tile_pool(name="eripool", bufs=5) as erip,
            tc.tile_pool(name="psA", bufs=2, space="PSUM") as psA,
            tc.tile_pool(name="psB", bufs=2, space="PSUM") as psB,
            tc.tile_pool(name="dram", bufs=1, space="DRAM") as dramp,
        ):
            # ---- constants: ONE hot DMA (128 big descriptors), cold on
            # the gpsimd queue; mom/ra after hot on sync ----
            cbig = constp.tile([128, CST_COLS], bf16, name="cbig")
            nc.sync.dma_start(cbig[:, 0:h1], cst_ext[:, 0:h1])
            nc.gpsimd.dma_start(cbig[:, h1:h2], cst_ext[:, h1:h2])
            mom_t = constp.tile([128, 1], f32, name="mom_t")
            nc.sync.dma_start(mom_t[:], mom_ext[:, :])
            nc.sync.dma_start(cbig[:, h2:], cst_ext[:, h2:])
            eps_t = constp.tile([128, 1], f32, name="eps_t")
            nc.gpsimd.memset(eps_t[:], 1e-30)
            rat = {}
            cst = {
                name: [cbig[0:rows, o:o + wdt] for (o, rows, wdt) in lst]
                for name, lst in cchunks.items()
            }

            # ---- dummy warmup AllReduce: absorbs the CC barrier and the
            # first-collective warmup penalty before the real ARs ----
            wu_in = dramp.tile([128, 64], bf16, name="wu_in")
            wu_out = dramp.tile([128, 64], bf16, name="wu_out",
                                addr_space="Shared")
            wu_s = constp.tile([128, 64], bf16, name="wu_s")
            nc.gpsimd.memset(wu_s[:], 0.0)
            with tc.high_priority():
                nc.gpsimd.dma_start(wu_in[:], wu_s[:])
                nc.gpsimd.collective_compute(
                    "AllReduce", Alu.add,
                    replica_groups=[list(range(N_CORES))],
                    ins=[wu_in.opt()], outs=[wu_out.opt()])

            # ---- collective bounces, partition-major [128, 2*KC] bf16 ----
            ar_in = {}
            ar_out = {}
            for c in range(C):
                for par in range(2):
                    ar_in[(c, par)] = dramp.tile(
                        [128, 2 * KC], bf16, name=f"ar_in_{c}_{par}")
                    ar_out[(c, par)] = dramp.tile(
                        [128, 2 * KC], bf16, name=f"ar_out_{c}_{par}",
                        addr_space="Shared")

            def blocked(ap):  # [m*128+p, j] dram view -> [p, m, j]
                return ap.rearrange("(m p) j -> p m j", p=128)

            Zt = {}         # (c, b) -> SBUF phase tile [128, 2, 4, KC]
            amp_t = {}      # accumulator chain for current channel
            tny = {}        # c -> [128, 4, 4] f32 nyquist vectors

            # ===== PASS 1 (per channel): forward + amp accumulation =====
            def emit_p1(c):
                for b in range(B_LOC):
                    xq = nc.scalar if (c == 0 and b < 3) else nc.sync
                    # host-precomputed radix-2 butterflies: rows (key,ch,p)
                    xp = workp.tile([128, 8, 256], bf16, name="xp", tag="xp")
                    xq.dma_start(xp[:], xp_ext[b, c].rearrange(
                        "(k p) j -> p k j", p=128))
                    # stage 1: 4 merged psum groups [128, 2(mp), 512]
                    ups = {}
                    for gi, (key, kidx, cn) in enumerate((
                        ("pE", 0, "CA"), ("pO", 1, "CB"),
                        ("mE", 2, "CA"), ("mO", 3, "CB"),
                    )):
                        pool = psA if gi % 2 == 0 else psB
                        ps = pool.tile([128, 2, 512], f32, name=f"ps1{key}",
                                       tag="pA" if gi % 2 == 0 else "pB")
                        for mp in range(2):
                            mps = slice(mp * 128, (mp + 1) * 128)
                            for ch in range(2):
                                nc.tensor.matmul(ps[:, mp, :],
                                                 xp[:, kidx * 2 + ch, mps],
                                                 cst[cn][ch][:],
                                                 start=(ch == 0), stop=(ch == 1))
                        u = stg2p.tile([128, 2, 512], bf16, name=f"u{key}",
                                       tag=f"u{key}")
                        if gi % 2 == 0:
                            nc.vector.tensor_copy(u[:], ps[:])
                        else:
                            nc.scalar.copy(u[:], ps[:])
                        ups[key] = u
                    # stage 2: per mh, merged [128, 2(dm), 512] psums
                    z = zp.tile([128, 2, NBLK, KC], bf16, name=f"z{c}{b}",
                                tag=f"z{c}{b}")
                    for mh in range(2):
                        sfx = "E" if mh == 0 else "O"
                        pze = psA.tile([128, 2, 512], f32, name="pze", tag="pA")
                        pzo = psB.tile([128, 2, 512], f32, name="pzo", tag="pB")
                        for dm in range(2):
                            mr = slice(dm * 128, dm * 128 + 128)
                            mi = slice(256 + dm * 128, 256 + dm * 128 + 128)
                            for mp in range(2):
                                nc.tensor.matmul(pze[:, dm, 0:2 * NE],
                                                 ups["p" + sfx][:, mp, mr],
                                                 cst["CE"][mp][:],
                                                 start=(mp == 0), stop=False)
                            for mp in range(2):
                                nc.tensor.matmul(pze[:, dm, 0:2 * NE],
                                                 ups["p" + sfx][:, mp, mi],
                                                 cst["CEm"][mp][:],
                                                 start=False, stop=(mp == 1))
                            for mp in range(2):
                                nc.tensor.matmul(pzo[:, dm, 0:2 * NO],
                                                 ups["m" + sfx][:, mp, mr],
                                                 cst["CO"][mp][:],
                                                 start=(mp == 0), stop=False)
                            for mp in range(2):
                                nc.tensor.matmul(pzo[:, dm, 0:2 * NO],
                                                 ups["m" + sfx][:, mp, mi],
                                                 cst["COm"][mp][:],
                                                 start=False, stop=(mp == 1))
                        ms2 = slice(2 * mh, 2 * mh + 2)
                        nc.vector.tensor_copy(z[:, 0, ms2, 0:NE], pze[:, :, 0:NE])
                        nc.scalar.copy(z[:, 1, ms2, 0:NE], pze[:, :, NE:2 * NE])
                        nc.vector.tensor_copy(z[:, 0, ms2, NE:KC], pzo[:, :, 0:NO])
                        nc.scalar.copy(z[:, 1, ms2, NE:KC], pzo[:, :, NO:2 * NO])
                    # amp chain, image-wide [128, 4, KC]
                    sq1 = w1p.tile([128, NBLK, KC], bf16, name="sq1", tag="sq1")
                    nc.gpsimd.tensor_tensor(sq1[:], z[:, 0], z[:, 0], Alu.mult)
                    sq2 = w1p.tile([128, NBLK, KC], bf16, name="sq2", tag="sq2")
                    nc.vector.tensor_tensor(sq2[:], z[:, 1], z[:, 1], Alu.mult)
                    a2 = workp.tile([128, NBLK, KC], bf16, name="a2", tag="a2")
                    nc.vector.tensor_tensor(a2[:], sq1[:], sq2[:], Alu.add)
                    lna = w1p.tile([128, NBLK, KC], f32, name="lna", tag="lna")
                    nc.scalar.activation(lna[:], a2[:], Act.Ln, bias=eps_t[:, 0:1])
                    ampb = workp.tile([128, NBLK, KC], bf16, name="amp",
                                      tag="amp")
                    nc.scalar.activation(ampb[:], lna[:], Act.Exp, scale=0.5)
                    inv = workp.tile([128, NBLK, KC], bf16, name="inv", tag="inv")
                    nc.scalar.activation(inv[:], lna[:], Act.Exp, scale=-0.5)
                    if b == 0:
                        amp_t["acc"] = ampb
                    elif b < B_LOC - 1:
                        acc = workp.tile([128, NBLK, KC], bf16, name="acc",
                                         tag="acc")
                        nc.gpsimd.tensor_tensor(acc[:], amp_t["acc"][:], ampb[:],
                                                Alu.add)
                        amp_t["acc"] = acc
                    else:
                        acast = w1p.tile([128, NBLK, KC], bf16, name=f"ac{c}",
                                         tag=f"ac{c}")
                        nc.vector.tensor_tensor(acast[:], amp_t["acc"][:],
                                                ampb[:], Alu.add)
                    # phase in place: Z *= 1/|Z|
                    nc.vector.tensor_tensor(z[:, 0], z[:, 0], inv[:], Alu.mult)
                    nc.vector.tensor_tensor(z[:, 1], z[:, 1], inv[:], Alu.mult)
                    Zt[(c, b)] = z
                with tc.high_priority():
                    for par in range(2):
                        nc.scalar.dma_start(
                            ar_in[(c, par)][:].rearrange("p (m j) -> p m j", m=2),
                            acast[:, 2 * par:2 * par + 2, :])
                        nc.gpsimd.collective_compute(
                            "AllReduce",
                            Alu.add,
                            replica_groups=[list(range(N_CORES))],
                            ins=[ar_in[(c, par)].opt()],
                            outs=[ar_out[(c, par)].opt()],
                        )

            # ===== PASS 2 (per channel): ratio + inverse =====
            def emit_p2(c):
                fe, fo = p2_floors[c]
                stk = tc.tile_wait_until(fe)
                stk.__enter__()
                rat[c] = constp.tile([128, NBLK, KC], bf16, name=f"rat{c}")
                nc.sync.dma_start(
                    rat[c][:].rearrange("p m j -> p (m j)"), ra_ext[c])
                # --- even-k_r phase: gated on AR (c, 0) only ---
                red_e = w1p.tile([128, 2, KC], bf16, name="red_e", tag="red_e")
                nc.sync.dma_start(
                    red_e[:], ar_out[(c, 0)][:].rearrange("p (m j) -> p m j", m=2))
                num_e = stgp.tile([128, 2, KC], bf16, name=f"nume_{c}",
                                  tag=f"nume_{c}")
                nc.vector.scalar_tensor_tensor(
                    num_e[:], red_e[:], mom_t[:, 0:1], rat[c][:, 0:2, :],
                    op0=Alu.mult, op1=Alu.add)
                twr_e = {}
                twi_e = {}
                eris = {}
                for b in range(B_LOC):
                    z = Zt[(c, b)]
                    tre = workp.tile([128, 2, KC], bf16, name="twre", tag="twre")
                    tie = workp.tile([128, 2, KC], bf16, name="twie", tag="twie")
                    nc.vector.tensor_tensor(tre[:], z[:, 0, 0:2, :], num_e[:],
                                            Alu.mult)
                    nc.vector.tensor_tensor(tie[:], z[:, 1, 0:2, :], num_e[:],
                                            Alu.mult)
                    twr_e[b], twi_e[b] = tre, tie
                    # stage-3 E chains for both q-slices, evac to SBUF
                    eb = []
                    for qi, (q0, q1) in enumerate(q_slices):
                        qs = slice(q0, q1)
                        pool = psA if qi == 0 else psB
                        pse = pool.tile([128, 512], f32, name="pse",
                                        tag="pA" if qi == 0 else "pB")
                        for k in range(2):
                            nc.tensor.matmul(pse[:], tre[:, k, qs],
                                             cst["MeRI"][k][:],
                                             start=(k == 0), stop=False)
                        for k in range(2):
                            nc.tensor.matmul(pse[:], tie[:, k, qs],
                                             cst["MeIR"][k][:],
                                             start=False, stop=(k == 1))
                        eri = erip.tile([128, 512], bf16, name="eri",
                                        tag=f"eri{qi}")
                        nc.scalar.copy(eri[:], pse[:])
                        eb.append(eri)
                    eris[b] = eb
                stk.__exit__(None, None, None)
                stk = tc.tile_wait_until(fo)
                stk.__enter__()
                # --- odd-k_r phase: gated on AR (c, 1) ---
                red_o = w1p.tile([128, 2, KC], bf16, name="red_o", tag="red_o")
                nc.sync.dma_start(
                    red_o[:], ar_out[(c, 1)][:].rearrange("p (m j) -> p m j", m=2))
                num_o = stgp.tile([128, 2, KC], bf16, name=f"numo_{c}",
                                  tag=f"numo_{c}")
                nc.vector.scalar_tensor_tensor(
                    num_o[:], red_o[:], mom_t[:, 0:1], rat[c][:, 2:4, :],
                    op0=Alu.mult, op1=Alu.add)
                # nyquist side: wn = P[:, :, :, nyq] * numer_nyq
                nq2 = w1p.tile([128, 2, 2, NBLK // 2, 1], bf16, name="nq2",
                               tag="nq2")
                for t in range(2):
                    nc.scalar.copy(nq2[:, 0, t], num_e[:, :, 128:129])
                    nc.scalar.copy(nq2[:, 1, t], num_o[:, :, 128:129])
                wn = w1p.tile([128, B_LOC, 2, NBLK, 1], bf16, name="wn", tag="wn")
                for b in range(B_LOC):
                    zs = Zt[(c, b)][:, :, :, 128:129]   # [128, 2(t), 4(m), 1]
                    nc.vector.tensor_tensor(wn[:, b, :, 0:2], zs[:, :, 0:2],
                                            nq2[:, 0], Alu.mult)
                    nc.vector.tensor_tensor(wn[:, b, :, 2:4], zs[:, :, 2:4],
                                            nq2[:, 1], Alu.mult)
                psn = psA.tile([128, NBLK, B_LOC], f32, name="psn", tag="pA")
                for m in range(NBLK):
                    ms = slice(m * 128, (m + 1) * 128)
                    for ti, cn in enumerate(("GnR", "GnI")):
                        for k in range(NBLK):
                            nc.tensor.matmul(
                                psn[:, m, :], cst[cn][k][:, ms],
                                wn[:, :, ti, k, 0],
                                start=(ti == 0 and k == 0),
                                stop=(ti == 1 and k == NBLK - 1))
                tnyT = stgp.tile([128, NBLK, B_LOC], f32, name=f"tny{c}",
                                 tag=f"tny{c}")
                nc.scalar.copy(tnyT[:], psn[:])
                tny[c] = tnyT
                for b in range(B_LOC):
                    z = Zt[(c, b)]
                    two = workp.tile([128, 2, KC], bf16, name="twro", tag="twro")
                    tio = workp.tile([128, 2, KC], bf16, name="twio", tag="twio")
                    nc.vector.tensor_tensor(two[:], z[:, 0, 2:4, :], num_o[:],
                                            Alu.mult)
                    nc.vector.tensor_tensor(tio[:], z[:, 1, 2:4, :], num_o[:],
                                            Alu.mult)
                    # stage-3 O chains + DIT butterfly vs the stored E part
                    t2 = []
                    for qi, (q0, q1) in enumerate(q_slices):
                        qs = slice(q0, q1)
                        pool = psA if qi == 0 else psB
                        pso3 = pool.tile([128, 512], f32, name="pso3",
                                         tag="pA" if qi == 0 else "pB")
                        for k in range(2):
                            nc.tensor.matmul(pso3[:], two[:, k, qs],
                                             cst["MoRI"][k][:],
                                             start=(k == 0), stop=False)
                        for k in range(2):
                            nc.tensor.matmul(pso3[:], tio[:, k, qs],
                                             cst["MoIR"][k][:],
                                             start=False, stop=(k == 1))
                        t2q = stg2p.tile([128, 2, 512], bf16, name=f"t2_{qi}",
                                         tag=f"t2_{qi}")
                        nc.vector.tensor_tensor(t2q[:, 0, :], eris[b][qi][:],
                                                pso3[:], Alu.add)
                        nc.vector.tensor_tensor(t2q[:, 1, :], eris[b][qi][:],
                                                pso3[:], Alu.subtract)
                        t2.append(t2q)
                    # stage 4 + nyquist rank-1 evac fold
                    ostg = workp.tile([128, NBLK, W], f32, name="ostg", tag="ostg")
                    for half in range(2):
                        pool = psA if half == 0 else psB
                        pso = pool.tile([128, 2, 512], f32, name="pso",
                                        tag="pA" if half == 0 else "pB")
                        for j in range(2):
                            m = 2 * half + j
                            msh = slice((m % 2) * 128, (m % 2) * 128 + 128)
                            msi = slice(256 + (m % 2) * 128,
                                        256 + (m % 2) * 128 + 128)
                            nc.tensor.matmul(pso[:, j, :], t2[0][:, half, msh],
                                             cst["Gw1"][0][:],
                                             start=True, stop=False)
                            nc.tensor.matmul(pso[:, j, :], t2[1][:, half, msh],
                                             cst["Gw1"][1][:],
                                             start=False, stop=False)
                            nc.tensor.matmul(pso[:, j, :], t2[0][:, half, msi],
                                             cst["Gw2"][0][:],
                                             start=False, stop=False)
                            nc.tensor.matmul(pso[:, j, :], t2[1][:, half, msi],
                                             cst["Gw2"][1][:],
                                             start=False, stop=True)
                        for j in range(2):
                            m = 2 * half + j
                            nc.vector.scalar_tensor_tensor(
                                ostg[:, m, :], cst["SGN"][0][:],
                                tny[c][:, m, b:b + 1], pso[:, j, :],
                                op0=Alu.mult, op1=Alu.add)
                    nc.gpsimd.dma_start(blocked(out_ext[b, c]), ostg[:])
                stk.__exit__(None, None, None)

            emit_p1(0)
            emit_p1(1)
            emit_p1(2)
            # virtual-time floors: park AR-gated pass-2 work after pass-1 in
            # every engine queue (the scheduler otherwise interleaves it early
            # and the real AllReduce latency head-of-line-blocks pass-1)
            p2_floors = {0: (0.300, 0.312), 1: (0.324, 0.336), 2: (0.360, 0.372)}
            emit_p2(0)
            emit_p2(1)
            emit_p2(2)
    nc.compile()
    return nc


def _host_inputs(x, running_amp):
    import ml_dtypes
    bf16 = ml_dtypes.bfloat16

    j = np.arange(H)
    theta = -2.0 * np.pi * np.outer(j, j) / H
    Fc = np.exp(1j * theta)           # F[n,k] = w^{nk}
    Gc = np.conj(Fc)                  # G[n,k] = w^{-nk}

    # stage1: B0/B1 = even/odd columns of F, top 256 rows; merged [r|i]
    B0 = Fc[0:256, 0::2]
    B1 = Fc[0:256, 1::2]
    CA = np.concatenate([B0.real, B0.imag], axis=1).astype(bf16)
    CB = np.concatenate([B1.real, B1.imag], axis=1).astype(bf16)

    # stage2: radix-2 DIF over n_c
    n = np.arange(256)[:, None]
    ke = np.arange(NE)[None, :]
    ko = np.arange(NO)[None, :]
    Be = np.exp(-2j * np.pi * n * (2 * ke) / H)
    Bo = np.exp(-2j * np.pi * n * (2 * ko + 1) / H)
    CE = np.concatenate([Be.real, Be.imag], axis=1).astype(bf16)
    CEm = np.concatenate([-Be.imag, Be.real], axis=1).astype(bf16)
    CO = np.concatenate([Bo.real, Bo.imag], axis=1).astype(bf16)
    COm = np.concatenate([-Bo.imag, Bo.real], axis=1).astype(bf16)

    # stage3 radix-2 DIT: even/odd columns of G restricted per derivation
    n256 = np.arange(256)
    Me = np.exp(2j * np.pi * np.outer(n256, n256) / 256.0)
    Mo = np.exp(2j * np.pi * np.outer(2 * n256 + 1, n256) / 512.0)
    MeRI = np.concatenate([Me.real, Me.imag], axis=1).astype(bf16)
    MeIR = np.concatenate([-Me.imag, Me.real], axis=1).astype(bf16)
    MoRI = np.concatenate([Mo.real, Mo.imag], axis=1).astype(bf16)
    MoIR = np.concatenate([-Mo.imag, Mo.real], axis=1).astype(bf16)

    # k_c column order: [evens incl nyq | odds]
    cols = np.concatenate([np.arange(0, 257, 2), np.arange(1, 256, 2)])
    cols_noq = np.concatenate([np.arange(0, 256, 2), np.arange(1, 256, 2)])

    # stage-4 weights: w=1 for k_c=0, else 2 (nyquist handled separately)
    wgt = np.where(cols_noq == 0, 1.0, 2.0)
    Gsel = Gc[cols_noq, :]
    Gw1 = (wgt[:, None] * Gsel.real / (H * W)).astype(bf16)
    Gw2 = (-wgt[:, None] * Gsel.imag / (H * W)).astype(bf16)

    perm_kr = np.concatenate([np.arange(0, H, 2), np.arange(1, H, 2)])
    # nyquist path: T2nyq = sum_kr G[n_r,k_r] W[k_r]; G symmetric; rows in
    # k_r-perm order; 1/N^2 and w=1 folded in. Re only: Gr*Wr - Gi*Wi.
    Gn = Gc[perm_kr, :]
    GnR = (Gn.real / (H * W)).astype(bf16)
    GnI = (-Gn.imag / (H * W)).astype(bf16)
    sgnrow = np.where(np.arange(W) % 2 == 0, 1.0, -1.0)
    SGN = np.broadcast_to(sgnrow, (128, W)).astype(bf16)

    cvals = {
        "CA": CA, "CB": CB, "CE": CE, "CEm": CEm, "CO": CO, "COm": COm,
        "MeRI": MeRI, "MeIR": MeIR, "MoRI": MoRI, "MoIR": MoIR,
        "Gw1": Gw1, "Gw2": Gw2, "GnR": GnR, "GnI": GnI, "SGN": SGN,
    }

    cchunks, cst_cols = _cst_layout()
    CST = np.zeros((128, cst_cols), bf16)
    for name, lst in cchunks.items():
        arr = cvals[name]
        p0 = 0
        for (o, rows, wdt) in lst:
            CST[0:rows, o:o + wdt] = arr[p0:p0 + rows, :]
            p0 += rows
    consts = {"CST": CST}

    if abs(float(running_amp.sum())) == 0.0:
        ra_half = np.zeros((C, H, KC), np.float32)
        mom_eff = 1.0 / B
    else:
        ra_s = np.fft.ifftshift(running_amp, axes=(-2, -1)).astype(np.float64)
        ra_rev = ra_s[:, (-np.arange(H)) % H][:, :, (-np.arange(W)) % W]
        ra_sym = (1.0 - MOMENTUM) * 0.5 * (ra_s + ra_rev)
        ra_half = ra_sym[:, perm_kr][:, :, cols].astype(np.float32)
        mom_eff = MOMENTUM / B
    mom = np.full((128, 1), mom_eff, np.float32)

    # host-side radix-2 DIF butterflies over both axes (device stage-1 lhsT):
    # rows (key, chunk, p) with key order [p0, p1, m0, m1]
    y0 = x[:, :, 0:256, :] + x[:, :, 256:512, :]
    y1 = x[:, :, 0:256, :] - x[:, :, 256:512, :]
    p0 = y0[..., 0:256] + y0[..., 256:512]
    m0 = y0[..., 0:256] - y0[..., 256:512]
    p1 = y1[..., 0:256] + y1[..., 256:512]
    m1 = y1[..., 0:256] - y1[..., 256:512]
    XP = np.stack([p0, p1, m0, m1], axis=2).reshape(B, C, 1024, 256).astype(bf16)

    # ra in partition-major layout [C, 128, NBLK*KC] (fewer DMA descriptors)
    ra_pm = ra_half.reshape(C, NBLK, 128, KC).transpose(0, 2, 1, 3).reshape(
        C, 128, NBLK * KC)

    in_maps = []
    for i in range(N_CORES):
        m = {"xp": np.ascontiguousarray(XP[i * B_LOC:(i + 1) * B_LOC]),
             "ra": ra_pm.astype(bf16), "mom": mom}
        m.update(consts)
        in_maps.append(m)
    return in_maps


def kernel(x: np.ndarray, running_amp: np.ndarray) -> np.ndarray:
    from concourse.bass_utils import run_bass_kernel_spmd

    if "nc" not in _cached:
        _cached["nc"] = _build()
    nc = _cached["nc"]
    in_maps = _host_inputs(np.asarray(x, np.float32),
                           np.asarray(running_amp, np.float32))
    res = run_bass_kernel_spmd(nc, in_maps, list(range(N_CORES)))
    out = np.concatenate([res.results[i]["out"] for i in range(N_CORES)], axis=0)
    return out.astype(np.float32)


# revision 16
# speedup vs baseline: 1.2221x; 1.0968x over previous
"""AmpNorm Trainium2 kernel: FFT-domain amplitude normalization (v4).

reference semantics:
    fft = fft2(x); amp = fftshift(|fft|); pha = angle(fft)
    amp_mean = mean(amp, axis=0)
    new_amp = (1-m)*running_amp + m*amp_mean     (EMA branch; init branch if sum==0)
    out = real(ifft2(ifftshift(new_amp) * exp(i*pha)))

Device formulation (per [512,512] image; shifts absorbed on host):
    Z = F @ X @ F computed for the k_c half-spectrum [0, 256] (x real =>
    Z Hermitian; the ratio is symmetrized host-side so the half determines
    the output exactly). P = Z/|Z| is stored IN SBUF (phase); out is rebuilt
    as ifft2 of numer * P where numer = ra_sym + mom * AllReduce(sum |Z|).

Stages (radix-2 split, every PE pass contracts <=256 rows):
  stage 1 (rows):   input butterflies precomputed ON HOST (xp tensor),
                    [B0|B0i]/[B1|B1i] merged-rhs matmuls.
  stage 2 (cols):   merged [Be_r|Be_i] (129 even k_c incl nyquist) and
                    [Bo_r|Bo_i] (128 odd k_c) matmuls.
  stage 3 (rows^-1): radix-2 DIT (Me/Mo on k_r parity chunks); E-part runs
                    early (gated on the even-k_r AllReduce half), O-part +
                    butterfly after the odd half.
  stage 4 (cols^-1): two-chunk contraction against Gw1/Gw2; nyquist rank-1
                    applied during PSUM evacuation via STT broadcast.

AllReduce is split per (channel, k_r-parity): 6 collectives of 131.5 KB,
so stage-3 E-work overlaps the odd-half wire time and the post-AR tail
shrinks. Bounce buffers are [128, 1028] partition-major (128 descriptors).
"""
import sys

sys.path.insert(0, "/opt/trn_rl_repo")

import numpy as np

N_CORES = 8
B, C, H, W = 32, 3, 512, 512
B_LOC = B // N_CORES          # 4 batches per core
N_IMG = B_LOC * C             # 12 images per core
NBLK = H // 128               # 4 partition blocks
KC = 257                      # half-spectrum cols: [evens 0..256 | odds]
NE = 129                      # even k_c count (incl nyquist)
NO = 128                      # odd k_c count
MOMENTUM = 0.1

# bf16 constants, packed column-wise into one [128, CST_COLS] tensor.
# name -> [rows, width]; order = DMA order (hot | cold).
CDEFS = {
    # hot -- stage1 merged rhs: CA = [B0r|B0i], CB = [B1r|B1i]
    "CA": [256, 512], "CB": [256, 512],
    # hot -- stage2 merged rhs (radix-2 DIF over n_c)
    "CE": [256, 2 * NE], "CEm": [256, 2 * NE],
    "CO": [256, 2 * NO], "COm": [256, 2 * NO],
    # cold -- pass-2 tables
    "MeRI": [256, 512], "MeIR": [256, 512],
    "MoRI": [256, 512], "MoIR": [256, 512],
    "Gw1": [256, W], "Gw2": [256, W],
    "GnR": [H, H], "GnI": [H, H],
    # (-1)^{n_c} broadcast row for the nyquist rank-1 evac fold
    "SGN": [128, W],
}


def _cst_layout():
    """name -> list of (col_offset, rows, width) per 128-row chunk."""
    chunks = {}
    off = 0
    for name, (r, wdt) in CDEFS.items():
        lst = []
        for p0 in range(0, r, 128):
            rows = min(128, r - p0)
            lst.append((off, rows, wdt))
            off += wdt
        chunks[name] = lst
    return chunks, off


_cached = {}


def _build():
    from concourse import bacc, tile, mybir

    f32 = mybir.dt.float32
    bf16 = mybir.dt.bfloat16
    Alu = mybir.AluOpType
    Act = mybir.ActivationFunctionType

    # Force every activation into the one table set covering
    # {copy, identity, square, ln, exp}: exactly one ACT table load.
    from concourse import hw_specs as _hw
    if not getattr(_hw, "_ampnorm_patched", False):
        _orig_get_tables = _hw.get_activation_tables

        def _patched(module_arch):
            tabs = _orig_get_tables(module_arch)
            keep = "natural_log_exp_and_others"
            covered = tabs[keep]
            return {
                name: (fns if name == keep else (fns - covered))
                for name, fns in tabs.items()
            }

        _hw.get_activation_tables = _patched
        _hw._ampnorm_patched = True
        import concourse.bacc as _bacc_mod
        _bacc_mod.get_activation_tables = _patched

    nc = bacc.Bacc("TRN2", target_bir_lowering=False, debug=False,
                   num_devices=N_CORES)

    xp_ext = nc.dram_tensor("xp", [B_LOC, C, 1024, 256], bf16,
                            kind="ExternalInput").ap()
    ra_ext = nc.dram_tensor("ra", [C, 128, NBLK * KC], bf16,
                            kind="ExternalInput").ap()
    mom_ext = nc.dram_tensor("mom", [128, 1], f32, kind="ExternalInput").ap()
    cchunks, CST_COLS = _cst_layout()
    cst_ext = nc.dram_tensor("CST", [128, CST_COLS], bf16,
                             kind="ExternalInput").ap()
    out_ext = nc.dram_tensor("out", [B_LOC, C, H, W], f32, kind="ExternalOutput").ap()

    h1 = cchunks["CE"][0][0]            # end of CB
    h2 = cchunks["MeRI"][0][0]          # end of COm (hot/cold split)

    q_slices = [(0, 128), (NE, KC)]     # stage3/4 k_c blocks

    with tile.TileContext(nc) as tc:
        with (
            tc.tile_pool(name="const", bufs=1) as constp,
            tc.tile_pool(name="zpool", bufs=1) as zp,
            tc.tile_pool(name="stage", bufs=1) as stgp,
            tc.tile_pool(name="stage2", bufs=2) as stg2p,
            tc.tile_pool(name="work", bufs=2) as workp,
            tc.tile_pool(name="work1", bufs=1) as w1p,
            tc.tile_pool(name="eripool", bufs=5) as erip,
            tc.tile_pool(name="psA", bufs=2, space="PSUM") as psA,
            tc.tile_pool(name="psB", bufs=2, space="PSUM") as psB,
            tc.tile_pool(name="dram", bufs=1, space="DRAM") as dramp,
        ):
            # ---- constants: ONE hot DMA (128 big descriptors), cold on
            # the gpsimd queue; mom/ra after hot on sync ----
            cbig = constp.tile([128, CST_COLS], bf16, name="cbig")
            nc.sync.dma_start(cbig[:, 0:h1], cst_ext[:, 0:h1])
            nc.gpsimd.dma_start(cbig[:, h1:h2], cst_ext[:, h1:h2])
            mom_t = constp.tile([128, 1], f32, name="mom_t")
            nc.sync.dma_start(mom_t[:], mom_ext[:, :])
            nc.sync.dma_start(cbig[:, h2:], cst_ext[:, h2:])
            eps_t = constp.tile([128, 1], f32, name="eps_t")
            nc.gpsimd.memset(eps_t[:], 1e-30)
            rat = {}
            cst = {
                name: [cbig[0:rows, o:o + wdt] for (o, rows, wdt) in lst]
                for name, lst in cchunks.items()
            }

            # ---- dummy warmup AllReduce: absorbs the CC barrier and the
            # first-collective warmup penalty before the real ARs ----
            wu_in = dramp.tile([128, 64], bf16, name="wu_in")
            wu_out = dramp.tile([128, 64], bf16, name="wu_out",
                                addr_space="Shared")
            wu_s = constp.tile([128, 64], bf16, name="wu_s")
            nc.gpsimd.memset(wu_s[:], 0.0)
            with tc.high_priority():
                nc.gpsimd.dma_start(wu_in[:], wu_s[:])
                nc.gpsimd.collective_compute(
                    "AllReduce", Alu.add,
                    replica_groups=[list(range(N_CORES))],
                    ins=[wu_in.opt()], outs=[wu_out.opt()])

            # ---- collective bounces, partition-major [128, 2*KC] bf16 ----
            ar_in = {}
            ar_out = {}
            for c in range(C):
                for par in range(2):
                    ar_in[(c, par)] = dramp.tile(
                        [128, 2 * KC], bf16, name=f"ar_in_{c}_{par}")
                    ar_out[(c, par)] = dramp.tile(
                        [128, 2 * KC], bf16, name=f"ar_out_{c}_{par}",
                        addr_space="Shared")

            def blocked(ap):  # [m*128+p, j] dram view -> [p, m, j]
                return ap.rearrange("(m p) j -> p m j", p=128)

            Zt = {}         # (c, b) -> SBUF phase tile [128, 2, 4, KC]
            amp_t = {}      # accumulator chain for current channel
            tny = {}        # c -> [128, 4, 4] f32 nyquist vectors

            # ===== PASS 1 (per channel): forward + amp accumulation =====
            def emit_p1(c):
                for b in range(B_LOC):
                    xq = nc.scalar if (c == 0 and b < 3) else nc.sync
                    # host-precomputed radix-2 butterflies: rows (key,ch,p)
                    xp = workp.tile([128, 8, 256], bf16, name="xp", tag="xp")
                    xq.dma_start(xp[:], xp_ext[b, c].rearrange(
                        "(k p) j -> p k j", p=128))
                    # stage 1: 4 merged psum groups [128, 2(mp), 512]
                    ups = {}
                    for gi, (key, kidx, cn) in enumerate((
                        ("pE", 0, "CA"), ("pO", 1, "CB"),
                        ("mE", 2, "CA"), ("mO", 3, "CB"),
                    )):
                        pool = psA if gi % 2 == 0 else psB
                        ps = pool.tile([128, 2, 512], f32, name=f"ps1{key}",
                                       tag="pA" if gi % 2 == 0 else "pB")
                        for mp in range(2):
                            mps = slice(mp * 128, (mp + 1) * 128)
                            for ch in range(2):
                                nc.tensor.matmul(ps[:, mp, :],
                                                 xp[:, kidx * 2 + ch, mps],
                                                 cst[cn][ch][:],
                                                 start=(ch == 0), stop=(ch == 1))
                        u = stg2p.tile([128, 2, 512], bf16, name=f"u{key}",
                                       tag=f"u{key}")
                        if gi == 0:
                            nc.vector.tensor_copy(u[:], ps[:])
                        else:
                            nc.scalar.copy(u[:], ps[:])
                        ups[key] = u
                    # stage 2: per mh, merged [128, 2(dm), 512] psums
                    z = zp.tile([128, 2, NBLK, KC], bf16, name=f"z{c}{b}",
                                tag=f"z{c}{b}")
                    for mh in range(2):
                        sfx = "E" if mh == 0 else "O"
                        pze = psA.tile([128, 2, 512], f32, name="pze", tag="pA")
                        pzo = psB.tile([128, 2, 512], f32, name="pzo", tag="pB")
                        for dm in range(2):
                            mr = slice(dm * 128, dm * 128 + 128)
                            mi = slice(256 + dm * 128, 256 + dm * 128 + 128)
                            for mp in range(2):
                                nc.tensor.matmul(pze[:, dm, 0:2 * NE],
                                                 ups["p" + sfx][:, mp, mr],
                                                 cst["CE"][mp][:],
                                                 start=(mp == 0), stop=False)
                            for mp in range(2):
                                nc.tensor.matmul(pze[:, dm, 0:2 * NE],
                                                 ups["p" + sfx][:, mp, mi],
                                                 cst["CEm"][mp][:],
                                                 start=False, stop=(mp == 1))
                            for mp in range(2):
                                nc.tensor.matmul(pzo[:, dm, 0:2 * NO],
                                                 ups["m" + sfx][:, mp, mr],
                                                 cst["CO"][mp][:],
                                                 start=(mp == 0), stop=False)
                            for mp in range(2):
                                nc.tensor.matmul(pzo[:, dm, 0:2 * NO],
                                                 ups["m" + sfx][:, mp, mi],
                                                 cst["COm"][mp][:],
                                                 start=False, stop=(mp == 1))
                        ms2 = slice(2 * mh, 2 * mh + 2)
                        nc.vector.tensor_copy(z[:, 0, ms2, 0:NE], pze[:, :, 0:NE])
                        nc.vector.tensor_copy(z[:, 1, ms2, 0:NE],
                                              pze[:, :, NE:2 * NE])
                        nc.vector.tensor_copy(z[:, 0, ms2, NE:KC], pzo[:, :, 0:NO])
                        nc.scalar.copy(z[:, 1, ms2, NE:KC], pzo[:, :, NO:2 * NO])
                    # amp chain, image-wide [128, 4, KC]; squares stay on V so
                    # the in-place phase muls never cross engines (WAR-free)
                    sq1 = workp.tile([128, NBLK, KC], bf16, name="sq1", tag="sq1")
                    nc.vector.tensor_tensor(sq1[:], z[:, 0], z[:, 0], Alu.mult)
                    sq2 = workp.tile([128, NBLK, KC], bf16, name="sq2", tag="sq2")
                    nc.vector.tensor_tensor(sq2[:], z[:, 1], z[:, 1], Alu.mult)
                    a2 = workp.tile([128, NBLK, KC], bf16, name="a2", tag="a2")
                    nc.vector.tensor_tensor(a2[:], sq1[:], sq2[:], Alu.add)
                    lna = workp.tile([128, NBLK, KC], f32, name="lna", tag="lna")
                    nc.scalar.activation(lna[:], a2[:], Act.Ln, bias=eps_t[:, 0:1])
                    ampb = workp.tile([128, NBLK, KC], bf16, name="amp",
                                      tag="amp")
                    nc.scalar.activation(ampb[:], lna[:], Act.Exp, scale=0.5)
                    inv = workp.tile([128, NBLK, KC], bf16, name="inv", tag="inv")
                    nc.scalar.activation(inv[:], lna[:], Act.Exp, scale=-0.5)
                    if b == 0:
                        amp_t["acc"] = ampb
                    elif b < B_LOC - 1:
                        acc = workp.tile([128, NBLK, KC], bf16, name="acc",
                                         tag="acc")
                        nc.gpsimd.tensor_tensor(acc[:], amp_t["acc"][:], ampb[:],
                                                Alu.add)
                        amp_t["acc"] = acc
                    else:
                        acast = w1p.tile([128, NBLK, KC], bf16, name=f"ac{c}",
                                         tag=f"ac{c}")
                        nc.vector.tensor_tensor(acast[:], amp_t["acc"][:],
                                                ampb[:], Alu.add)
                    # phase in place: Z *= 1/|Z|
                    nc.vector.tensor_tensor(z[:, 0], z[:, 0], inv[:], Alu.mult)
                    nc.vector.tensor_tensor(z[:, 1], z[:, 1], inv[:], Alu.mult)
                    Zt[(c, b)] = z
                with tc.high_priority():
                    for par in range(2):
                        nc.scalar.dma_start(
                            ar_in[(c, par)][:].rearrange("p (m j) -> p m j", m=2),
                            acast[:, 2 * par:2 * par + 2, :])
                        nc.gpsimd.collective_compute(
                            "AllReduce",
                            Alu.add,
                            replica_groups=[list(range(N_CORES))],
                            ins=[ar_in[(c, par)].opt()],
                            outs=[ar_out[(c, par)].opt()],
                        )

            # ===== PASS 2 (per channel): ratio + inverse =====
            def emit_p2(c):
                fe, fo = p2_floors[c]
                stk = tc.tile_wait_until(fe)
                stk.__enter__()
                rat[c] = constp.tile([128, NBLK, KC], bf16, name=f"rat{c}")
                nc.sync.dma_start(
                    rat[c][:].rearrange("p m j -> p (m j)"), ra_ext[c])
                # --- even-k_r phase: gated on AR (c, 0) only ---
                red_e = w1p.tile([128, 2, KC], bf16, name="red_e", tag="red_e")
                nc.sync.dma_start(
                    red_e[:], ar_out[(c, 0)][:].rearrange("p (m j) -> p m j", m=2))
                num_e = stgp.tile([128, 2, KC], bf16, name=f"nume_{c}",
                                  tag=f"nume_{c}")
                nc.vector.scalar_tensor_tensor(
                    num_e[:], red_e[:], mom_t[:, 0:1], rat[c][:, 0:2, :],
                    op0=Alu.mult, op1=Alu.add)
                twr_e = {}
                twi_e = {}
                eris = {}
                for b in range(B_LOC):
                    z = Zt[(c, b)]
                    tre = workp.tile([128, 2, KC], bf16, name="twre", tag="twre")
                    tie = workp.tile([128, 2, KC], bf16, name="twie", tag="twie")
                    nc.vector.tensor_tensor(tre[:], z[:, 0, 0:2, :], num_e[:],
                                            Alu.mult)
                    nc.vector.tensor_tensor(tie[:], z[:, 1, 0:2, :], num_e[:],
                                            Alu.mult)
                    twr_e[b], twi_e[b] = tre, tie
                    # stage-3 E chains for both q-slices, evac to SBUF
                    eb = []
                    for qi, (q0, q1) in enumerate(q_slices):
                        qs = slice(q0, q1)
                        pool = psA if qi == 0 else psB
                        pse = pool.tile([128, 512], f32, name="pse",
                                        tag="pA" if qi == 0 else "pB")
                        for k in range(2):
                            nc.tensor.matmul(pse[:], tre[:, k, qs],
                                             cst["MeRI"][k][:],
                                             start=(k == 0), stop=False)
                        for k in range(2):
                            nc.tensor.matmul(pse[:], tie[:, k, qs],
                                             cst["MeIR"][k][:],
                                             start=False, stop=(k == 1))
                        eri = erip.tile([128, 512], bf16, name="eri",
                                        tag=f"eri{qi}")
                        nc.scalar.copy(eri[:], pse[:])
                        eb.append(eri)
                    eris[b] = eb
                stk.__exit__(None, None, None)
                stk = tc.tile_wait_until(fo)
                stk.__enter__()
                # --- odd-k_r phase: gated on AR (c, 1) ---
                red_o = w1p.tile([128, 2, KC], bf16, name="red_o", tag="red_o")
                nc.sync.dma_start(
                    red_o[:], ar_out[(c, 1)][:].rearrange("p (m j) -> p m j", m=2))
                num_o = stgp.tile([128, 2, KC], bf16, name=f"numo_{c}",
                                  tag=f"numo_{c}")
                nc.vector.scalar_tensor_tensor(
                    num_o[:], red_o[:], mom_t[:, 0:1], rat[c][:, 2:4, :],
                    op0=Alu.mult, op1=Alu.add)
                # nyquist side: wn = P[:, :, :, nyq] * numer_nyq
                nq2 = w1p.tile([128, 2, 2, NBLK // 2, 1], bf16, name="nq2",
                               tag="nq2")
                for t in range(2):
                    nc.scalar.copy(nq2[:, 0, t], num_e[:, :, 128:129])
                    nc.scalar.copy(nq2[:, 1, t], num_o[:, :, 128:129])
                wn = w1p.tile([128, B_LOC, 2, NBLK, 1], bf16, name="wn", tag="wn")
                for b in range(B_LOC):
                    zs = Zt[(c, b)][:, :, :, 128:129]   # [128, 2(t), 4(m), 1]
                    nc.vector.tensor_tensor(wn[:, b, :, 0:2], zs[:, :, 0:2],
                                            nq2[:, 0], Alu.mult)
                    nc.vector.tensor_tensor(wn[:, b, :, 2:4], zs[:, :, 2:4],
                                            nq2[:, 1], Alu.mult)
                psn = psA.tile([128, NBLK, B_LOC], f32, name="psn", tag="pA")
                for m in range(NBLK):
                    ms = slice(m * 128, (m + 1) * 128)
                    for ti, cn in enumerate(("GnR", "GnI")):
                        for k in range(NBLK):
                            nc.tensor.matmul(
                                psn[:, m, :], cst[cn][k][:, ms],
                                wn[:, :, ti, k, 0],
                                start=(ti == 0 and k == 0),
                                stop=(ti == 1 and k == NBLK - 1))
                tnyT = stgp.tile([128, NBLK, B_LOC], f32, name=f"tny{c}",
                                 tag=f"tny{c}")
                nc.scalar.copy(tnyT[:], psn[:])
                tny[c] = tnyT
                for b in range(B_LOC):
                    z = Zt[(c, b)]
                    two = workp.tile([128, 2, KC], bf16, name="twro", tag="twro")
                    tio = workp.tile([128, 2, KC], bf16, name="twio", tag="twio")
                    nc.vector.tensor_tensor(two[:], z[:, 0, 2:4, :], num_o[:],
                                            Alu.mult)
                    nc.vector.tensor_tensor(tio[:], z[:, 1, 2:4, :], num_o[:],
                                            Alu.mult)
                    # stage-3 O chains + DIT butterfly vs the stored E part
                    t2 = []
                    for qi, (q0, q1) in enumerate(q_slices):
                        qs = slice(q0, q1)
                        pool = psA if qi == 0 else psB
                        pso3 = pool.tile([128, 512], f32, name="pso3",
                                         tag="pA" if qi == 0 else "pB")
                        for k in range(2):
                            nc.tensor.matmul(pso3[:], two[:, k, qs],
                                             cst["MoRI"][k][:],
                                             start=(k == 0), stop=False)
                        for k in range(2):
                            nc.tensor.matmul(pso3[:], tio[:, k, qs],
                                             cst["MoIR"][k][:],
                                             start=False, stop=(k == 1))
                        t2q = stg2p.tile([128, 2, 512], bf16, name=f"t2_{qi}",
                                         tag=f"t2_{qi}")
                        nc.vector.tensor_tensor(t2q[:, 0, :], eris[b][qi][:],
                                                pso3[:], Alu.add)
                        nc.vector.tensor_tensor(t2q[:, 1, :], eris[b][qi][:],
                                                pso3[:], Alu.subtract)
                        t2.append(t2q)
                    # stage 4 + nyquist rank-1 evac fold
                    ostg = workp.tile([128, NBLK, W], f32, name="ostg", tag="ostg")
                    for half in range(2):
                        pool = psA if half == 0 else psB
                        pso = pool.tile([128, 2, 512], f32, name="pso",
                                        tag="pA" if half == 0 else "pB")
                        for j in range(2):
                            m = 2 * half + j
                            msh = slice((m % 2) * 128, (m % 2) * 128 + 128)
                            msi = slice(256 + (m % 2) * 128,
                                        256 + (m % 2) * 128 + 128)
                            nc.tensor.matmul(pso[:, j, :], t2[0][:, half, msh],
                                             cst["Gw1"][0][:],
                                             start=True, stop=False)
                            nc.tensor.matmul(pso[:, j, :], t2[1][:, half, msh],
                                             cst["Gw1"][1][:],
                                             start=False, stop=False)
                            nc.tensor.matmul(pso[:, j, :], t2[0][:, half, msi],
                                             cst["Gw2"][0][:],
                                             start=False, stop=False)
                            nc.tensor.matmul(pso[:, j, :], t2[1][:, half, msi],
                                             cst["Gw2"][1][:],
                                             start=False, stop=True)
                        for j in range(2):
                            m = 2 * half + j
                            nc.vector.scalar_tensor_tensor(
                                ostg[:, m, :], cst["SGN"][0][:],
                                tny[c][:, m, b:b + 1], pso[:, j, :],
                                op0=Alu.mult, op1=Alu.add)
                    nc.gpsimd.dma_start(blocked(out_ext[b, c]), ostg[:])
                stk.__exit__(None, None, None)

            emit_p1(0)
            emit_p1(1)
            emit_p1(2)
            # virtual-time floors: park AR-gated pass-2 work after pass-1 in
            # every engine queue (the scheduler otherwise interleaves it early
            # and the real AllReduce latency head-of-line-blocks pass-1)
            p2_floors = {0: (0.300, 0.312), 1: (0.324, 0.336), 2: (0.360, 0.372)}
            emit_p2(0)
            emit_p2(1)
            emit_p2(2)
    nc.compile()
    return nc


def _host_inputs(x, running_amp):
    import ml_dtypes
    bf16 = ml_dtypes.bfloat16

    j = np.arange(H)
    theta = -2.0 * np.pi * np.outer(j, j) / H
    Fc = np.exp(1j * theta)           # F[n,k] = w^{nk}
    Gc = np.conj(Fc)                  # G[n,k] = w^{-nk}

    # stage1: B0/B1 = even/odd columns of F, top 256 rows; merged [r|i]
    B0 = Fc[0:256, 0::2]
    B1 = Fc[0:256, 1::2]
    CA = np.concatenate([B0.real, B0.imag], axis=1).astype(bf16)
    CB = np.concatenate([B1.real, B1.imag], axis=1).astype(bf16)

    # stage2: radix-2 DIF over n_c
    n = np.arange(256)[:, None]
    ke = np.arange(NE)[None, :]
    ko = np.arange(NO)[None, :]
    Be = np.exp(-2j * np.pi * n * (2 * ke) / H)
    Bo = np.exp(-2j * np.pi * n * (2 * ko + 1) / H)
    CE = np.concatenate([Be.real, Be.imag], axis=1).astype(bf16)
    CEm = np.concatenate([-Be.imag, Be.real], axis=1).astype(bf16)
    CO = np.concatenate([Bo.real, Bo.imag], axis=1).astype(bf16)
    COm = np.concatenate([-Bo.imag, Bo.real], axis=1).astype(bf16)

    # stage3 radix-2 DIT: even/odd columns of G restricted per derivation
    n256 = np.arange(256)
    Me = np.exp(2j * np.pi * np.outer(n256, n256) / 256.0)
    Mo = np.exp(2j * np.pi * np.outer(2 * n256 + 1, n256) / 512.0)
    MeRI = np.concatenate([Me.real, Me.imag], axis=1).astype(bf16)
    MeIR = np.concatenate([-Me.imag, Me.real], axis=1).astype(bf16)
    MoRI = np.concatenate([Mo.real, Mo.imag], axis=1).astype(bf16)
    MoIR = np.concatenate([-Mo.imag, Mo.real], axis=1).astype(bf16)

    # k_c column order: [evens incl nyq | odds]
    cols = np.concatenate([np.arange(0, 257, 2), np.arange(1, 256, 2)])
    cols_noq = np.concatenate([np.arange(0, 256, 2), np.arange(1, 256, 2)])

    # stage-4 weights: w=1 for k_c=0, else 2 (nyquist handled separately)
    wgt = np.where(cols_noq == 0, 1.0, 2.0)
    Gsel = Gc[cols_noq, :]
    Gw1 = (wgt[:, None] * Gsel.real / (H * W)).astype(bf16)
    Gw2 = (-wgt[:, None] * Gsel.imag / (H * W)).astype(bf16)

    perm_kr = np.concatenate([np.arange(0, H, 2), np.arange(1, H, 2)])
    # nyquist path: T2nyq = sum_kr G[n_r,k_r] W[k_r]; G symmetric; rows in
    # k_r-perm order; 1/N^2 and w=1 folded in. Re only: Gr*Wr - Gi*Wi.
    Gn = Gc[perm_kr, :]
    GnR = (Gn.real / (H * W)).astype(bf16)
    GnI = (-Gn.imag / (H * W)).astype(bf16)
    sgnrow = np.where(np.arange(W) % 2 == 0, 1.0, -1.0)
    SGN = np.broadcast_to(sgnrow, (128, W)).astype(bf16)

    cvals = {
        "CA": CA, "CB": CB, "CE": CE, "CEm": CEm, "CO": CO, "COm": COm,
        "MeRI": MeRI, "MeIR": MeIR, "MoRI": MoRI, "MoIR": MoIR,
        "Gw1": Gw1, "Gw2": Gw2, "GnR": GnR, "GnI": GnI, "SGN": SGN,
    }

    cchunks, cst_cols = _cst_layout()
    CST = np.zeros((128, cst_cols), bf16)
    for name, lst in cchunks.items():
        arr = cvals[name]
        p0 = 0
        for (o, rows, wdt) in lst:
            CST[0:rows, o:o + wdt] = arr[p0:p0 + rows, :]
            p0 += rows
    consts = {"CST": CST}

    if abs(float(running_amp.sum())) == 0.0:
        ra_half = np.zeros((C, H, KC), np.float32)
        mom_eff = 1.0 / B
    else:
        ra_s = np.fft.ifftshift(running_amp, axes=(-2, -1)).astype(np.float64)
        ra_rev = ra_s[:, (-np.arange(H)) % H][:, :, (-np.arange(W)) % W]
        ra_sym = (1.0 - MOMENTUM) * 0.5 * (ra_s + ra_rev)
        ra_half = ra_sym[:, perm_kr][:, :, cols].astype(np.float32)
        mom_eff = MOMENTUM / B
    mom = np.full((128, 1), mom_eff, np.float32)

    # host-side radix-2 DIF butterflies over both axes (device stage-1 lhsT):
    # rows (key, chunk, p) with key order [p0, p1, m0, m1]
    y0 = x[:, :, 0:256, :] + x[:, :, 256:512, :]
    y1 = x[:, :, 0:256, :] - x[:, :, 256:512, :]
    p0 = y0[..., 0:256] + y0[..., 256:512]
    m0 = y0[..., 0:256] - y0[..., 256:512]
    p1 = y1[..., 0:256] + y1[..., 256:512]
    m1 = y1[..., 0:256] - y1[..., 256:512]
    XP = np.stack([p0, p1, m0, m1], axis=2).reshape(B, C, 1024, 256).astype(bf16)

    # ra in partition-major layout [C, 128, NBLK*KC] (fewer DMA descriptors)
    ra_pm = ra_half.reshape(C, NBLK, 128, KC).transpose(0, 2, 1, 3).reshape(
        C, 128, NBLK * KC)

    in_maps = []
    for i in range(N_CORES):
        m = {"xp": np.ascontiguousarray(XP[i * B_LOC:(i + 1) * B_LOC]),
             "ra": ra_pm.astype(bf16), "mom": mom}
        m.update(consts)
        in_maps.append(m)
    return in_maps


def kernel(x: np.ndarray, running_amp: np.ndarray) -> np.ndarray:
    from concourse.bass_utils import run_bass_kernel_spmd

    if "nc" not in _cached:
        _cached["nc"] = _build()
    nc = _cached["nc"]
    in_maps = _host_inputs(np.asarray(x, np.float32),
                           np.asarray(running_amp, np.float32))
    res = run_bass_kernel_spmd(nc, in_maps, list(range(N_CORES)))
    out = np.concatenate([res.results[i]["out"] for i in range(N_CORES)], axis=0)
    return out.astype(np.float32)
